# revision 1
# baseline (speedup 1.0000x reference)
"""Depthwise 3x3 conv (stride 1, SAME, depth_multiplier 1) on 8 trn2 NeuronCores.

Input  x [32, 112, 112, 192] f32, w [3, 3, 1, 192] f32, b [192] f32.
Output [32, 112, 112, 192] f32.

Strategy (pure data parallelism, batch sharded 4 images/core):
  SBUF layout: partitions = h (112 rows), free = (w, c) with one zero guard
  w-slot on each side (114 x 192 elements, bf16).
  - ScalarE casts fp32 input chunks to bf16 into the image tile.
  - VectorE computes the 9 tap products per window: prod_t = x(w+dw) * W[t,c]
    (tensor_tensor mult vs a broadcast weight tile, bf16 2x mode).
  - TensorE applies the h-shifts and sums all taps + bias into PSUM (fp32):
    psum[:, chunk] += S_dh.T @ prod_t; S matrices are 0/1 shift matrices that
    also implement SAME padding at h edges; a 10th matmul adds the bias row.
  - ScalarE evacuates PSUM -> SBUF fp32; HWDGE DMA writes NHWC output.
"""
import dataclasses

import numpy as np
import ml_dtypes

import concourse.bacc as bacc
import concourse.mybir as mybir
from concourse.bass_utils import run_bass_kernel_spmd
from concourse.tile import TileContext

F32 = mybir.dt.float32
BF16 = mybir.dt.bfloat16

B, H, W, C = 32, 112, 112, 192
N_CORES = 8
B_SH = B // N_CORES          # images per core
WWIN = 8                     # w columns per window
PCH = 512                    # PSUM chunk (one bank of fp32)


class _Geom:
    def __init__(self, h=H, w=W, c=C):
        self.h, self.w, self.c = h, w, c
        self.wwin = WWIN
        self.nwin = w // self.wwin
        self.wfree = self.wwin * c
        self.nch = self.wfree // PCH
        self.wg = w + 2
        self.xfree = self.wg * c


def _alloc_tiles(nc, tc, g, cpool, xpool, ppool, wb_compress=False,
                 fuse_mults=False, prod_sets=2):
    if wb_compress:
        wb = cpool.tile([128, 9 * g.c + g.wfree], BF16, tag="wb", name="wb")
    else:
        wb = cpool.tile([128, 10 * g.wfree], BF16, tag="wb", name="wb")
    sm = cpool.tile([128, 4 * 128], BF16, tag="sm", name="sm")
    xts = [xpool.tile([g.h, g.xfree], BF16, tag=f"x{i}", name=f"x{i}")
           for i in range(2)]
    for xt in xts:
        nc.vector.memset(xt[:, 0:g.c], 0.0)
        nc.vector.memset(xt[:, (g.wg - 1) * g.c:g.wg * g.c], 0.0)
    if fuse_mults == "all":
        prods = [[ppool.tile([128, 9 * g.wfree], BF16, tag=f"p{s}_0",
                             name=f"p{s}_0")] for s in range(2)]
    elif fuse_mults:
        prods = [[ppool.tile([128, 3 * g.wfree], BF16, tag=f"p{s}_{t}",
                             name=f"p{s}_{t}") for t in range(3)]
                 for s in range(prod_sets)]
    else:
        prods = [[ppool.tile([128, g.wfree], BF16, tag=f"p{s}_{t}",
                             name=f"p{s}_{t}") for t in range(9)]
                 for s in range(2)]
    for pset in prods:
        for p in pset:
            nc.vector.memset(p[:, :], 0.0)
    return wb, sm, xts, prods


def _emit_image(nc, g, wb, sm, xt, prods, spool, opool, pspool, x_img, y_img,
                skip=(), opts=None):
    """Emit load+compute+store for one image. x_img/y_img: [h, w, c] DRAM APs."""
    opts = opts or {}
    lc = opts.get("load_chunk", 1)        # windows per load DMA
    mm_order = opts.get("mm_order", "dh")
    wbc = opts.get("wb_compress", True)
    fm = opts.get("fuse_mults", False)
    bias = opts.get("bias", True)
    c, wwin, wfree, nch = g.c, g.wwin, g.wfree, g.nch
    for wd in range(0, g.nwin, lc):
        if "load" in skip:
            break
        w0 = wd * wwin
        stg = spool.tile([g.h, wfree * lc], F32, tag="stg", name="stg")
        nc.sync.dma_start(
            out=stg[:, :],
            in_=x_img[:, w0:w0 + wwin * lc, :].rearrange("h w c -> h (w c)"))
        nc.scalar.activation(
            xt[:, (1 + w0) * c:(1 + w0 + wwin * lc) * c], stg[:, :],
            mybir.ActivationFunctionType.Copy)
    for wd in range(g.nwin):
        w0 = wd * wwin
        pset = prods[wd % len(prods)]
        if fm == "all" and "mult" not in skip:
            out5 = pset[0][:g.h, :].rearrange(
                "h (a s w c) -> h a s w c", a=3, s=3, w=wwin, c=c)
            in0 = xt[:, w0 * c:w0 * c + wfree]
            in0 = dataclasses.replace(
                in0, ap=[[g.xfree, g.h], [0, 3], [c, 3], [c, wwin], [1, c]])
            in1 = (wb[:g.h, 0:9 * c]
                   .rearrange("h (a s c) -> h a s c", a=3, c=c).unsqueeze(3)
                   .broadcast_to([g.h, 3, 3, wwin, c]))
            nc.vector.tensor_mul(out5, in0, in1)
        for dh in range(3):
            if "mult" in skip or fm == "all":
                break
            if fm:
                # one op: 3 overlapping dw-segments x broadcast weights
                out4 = pset[dh][:g.h, :].rearrange(
                    "h (s w c) -> h s w c", c=c, w=wwin)
                in0 = xt[:, w0 * c:w0 * c + wfree]
                in0 = dataclasses.replace(
                    in0, ap=[[g.xfree, g.h], [c, 3], [c, wwin], [1, c]])
                in1 = (wb[:g.h, dh * 3 * c:(dh * 3 + 3) * c]
                       .rearrange("h (s c) -> h s c", c=c).unsqueeze(2)
                       .broadcast_to([g.h, 3, wwin, c]))
                nc.vector.tensor_mul(out4, in0, in1)
                continue
            for dw in range(3):
                t = dh * 3 + dw
                off = (w0 + dw) * c
                if wbc:
                    nc.vector.tensor_mul(
                        pset[t][:g.h, :].rearrange("h (w c) -> h w c", c=c),
                        xt[:, off:off + wfree].rearrange("h (w c) -> h w c", c=c),
                        wb[:g.h, t * c:(t + 1) * c].unsqueeze(1)
                        .broadcast_to([g.h, wwin, c]))
                else:
                    nc.vector.tensor_mul(
                        pset[t][:g.h, :],
                        xt[:, off:off + wfree],
                        wb[:g.h, t * wfree:(t + 1) * wfree])
        if "pe" in skip:
            continue
        ps = pspool.tile([128, wfree], F32, tag="ps", name="ps")
        def _sl(ch):
            return slice(ch * PCH, (ch + 1) * PCH)
        def _rhs(dh, dw, ch):
            if fm == "all":
                t = dh * 3 + dw
                return pset[0][:, t * wfree + ch * PCH:t * wfree + (ch + 1) * PCH]
            if fm:
                return pset[dh][:, dw * wfree + ch * PCH:
                                dw * wfree + (ch + 1) * PCH]
            return pset[dh * 3 + dw][:, _sl(ch)]
        if mm_order == "ch":
            for ch in range(nch):
                sl = _sl(ch)
                for dh in range(3):
                    for dw in range(3):
                        nc.tensor.matmul(
                            ps[:, sl], sm[:, dh * 128:(dh + 1) * 128],
                            _rhs(dh, dw, ch),
                            start=(dh == 0 and dw == 0),
                            stop=(not bias and dh == 2 and dw == 2))
                if bias:
                    bias0 = 9 * c if wbc else 9 * wfree
                    nc.tensor.matmul(
                        ps[:, sl], sm[:, 3 * 128:4 * 128],
                        wb[:, bias0 + ch * PCH:bias0 + (ch + 1) * PCH],
                        start=False, stop=True)
        else:  # dh-major: long same-stationary runs
            for dh in range(3):
                for ch in range(nch):
                    sl = _sl(ch)
                    for dw in range(3):
                        nc.tensor.matmul(
                            ps[:, sl], sm[:, dh * 128:(dh + 1) * 128],
                            _rhs(dh, dw, ch),
                            start=(dh == 0 and dw == 0),
                            stop=(not bias and dh == 2 and dw == 2))
            if bias:
                bias0 = 9 * c if wbc else 9 * wfree
                for ch in range(nch):
                    sl = _sl(ch)
                    nc.tensor.matmul(
                        ps[:, sl], sm[:, 3 * 128:4 * 128],
                        wb[:, bias0 + ch * PCH:bias0 + (ch + 1) * PCH],
                        start=False, stop=True)
        outc = opool.tile([g.h, wfree], F32, tag="outc", name="outc")
        nc.scalar.activation(outc[:, :], ps[:g.h, :],
                             mybir.ActivationFunctionType.Copy)
        nc.sync.dma_start(
            out=y_img[:, w0:w0 + wwin, :].rearrange("h w c -> h (w c)"),
            in_=outc[:, :])


def _pools(nc, tc):
    return (
        tc.tile_pool(name="const", bufs=1),
        tc.tile_pool(name="xp", bufs=1),
        tc.tile_pool(name="prodp", bufs=1),
        tc.tile_pool(name="stg", bufs=2),
        tc.tile_pool(name="outp", bufs=3),
        tc.tile_pool(name="psum", bufs=2, space="PSUM"),
    )


def _build_module(b_sh=B_SH, h=H, w=W, c=C, opts=None):
    g = _Geom(h, w, c)
    opts = dict(_DEFAULT_OPTS, **(opts or {}))
    nc = bacc.Bacc("TRN2")
    x = nc.dram_tensor("x", [b_sh, h, w, c], F32, kind="ExternalInput")
    nwb = 9 * c + g.wfree if opts.get("wb_compress") else 10 * g.wfree
    wbias = nc.dram_tensor("wbias", [128, nwb], BF16,
                           kind="ExternalInput")
    smats = nc.dram_tensor("smats", [128, 4 * 128], BF16, kind="ExternalInput")
    y = nc.dram_tensor("y", [b_sh, h, w, c], F32, kind="ExternalOutput")

    with TileContext(nc) as tc:
        with (
            tc.tile_pool(name="const", bufs=1) as cpool,
            tc.tile_pool(name="xp", bufs=1) as xpool,
            tc.tile_pool(name="prodp", bufs=1) as ppool,
            tc.tile_pool(name="stg", bufs=opts.get("stg_bufs", 2)) as spool,
            tc.tile_pool(name="outp", bufs=opts.get("out_bufs", 3)) as opool,
            tc.tile_pool(name="psum", bufs=2, space="PSUM") as pspool,
        ):
            wb, sm, xts, prods = _alloc_tiles(nc, tc, g, cpool, xpool, ppool,
                                              opts.get("wb_compress", False),
                                              opts.get("fuse_mults", False),
                                              opts.get("prod_sets", 2))
            nc.sync.dma_start(out=wb[:, :], in_=wbias[:, :])
            nc.sync.dma_start(out=sm[:, :], in_=smats[:, :])
            for img in range(b_sh):
                _emit_image(nc, g, wb, sm, xts[img % 2], prods,
                            spool, opool, pspool, x[img], y[img], opts=opts)
    nc.compile()
    return nc


def _build_timing_module(h=H, w=W, c=C, iters=8, skip=(), opts=None):
    """Same per-image pipeline in a HW loop over internal DRAM tensors.

    One loop iteration = 2 image passes (ping-pong tiles). No host IO.
    """
    g = _Geom(h, w, c)
    opts = dict(_DEFAULT_OPTS, **(opts or {}))
    nc = bacc.Bacc("TRN2")
    x = nc.dram_tensor("xg", [2, h, w, c], F32)
    y = nc.dram_tensor("yg", [2, h, w, c], F32)
    yo = nc.dram_tensor("yo", [1, 8], F32, kind="ExternalOutput")

    with TileContext(nc) as tc:
        with (
            tc.tile_pool(name="const", bufs=1) as cpool,
            tc.tile_pool(name="xp", bufs=1) as xpool,
            tc.tile_pool(name="prodp", bufs=1) as ppool,
            tc.tile_pool(name="stg", bufs=opts.get("stg_bufs", 2)) as spool,
            tc.tile_pool(name="outp", bufs=opts.get("out_bufs", 3)) as opool,
            tc.tile_pool(name="psum", bufs=2, space="PSUM") as pspool,
        ):
            wb, sm, xts, prods = _alloc_tiles(nc, tc, g, cpool, xpool, ppool,
                                              opts.get("wb_compress", False),
                                              opts.get("fuse_mults", False),
                                              opts.get("prod_sets", 2))
            nc.vector.memset(wb[:, :], 0.01)
            nc.vector.memset(sm[:, :], 0.0)
            # zero the source so bf16 garbage can't produce NaNs
            zt = spool.tile([g.h, g.wfree], F32, tag="stg", name="zt")
            nc.vector.memset(zt[:, :], 0.5)
            for img in range(2):
                for wd in range(g.nwin):
                    nc.sync.dma_start(
                        out=x[img, :, wd * g.wwin:(wd + 1) * g.wwin, :]
                        .rearrange("h w c -> h (w c)"),
                        in_=zt[:, :])
            with tc.For_i(0, iters) as _:
                for img in range(2):
                    _emit_image(nc, g, wb, sm, xts[img], prods,
                                spool, opool, pspool, x[img], y[img], skip=skip,
                                opts=opts)
            of = opool.tile([1, 8], F32, tag="outc", name="of")
            nc.vector.memset(of[:, :], 0.0)
            nc.sync.dma_start(out=yo[:, :], in_=of[:1, :8])
    nc.compile()
    return nc


def _host_consts(wk, bk, h=H, w=W, c=C, wb_compress=False):
    """wk [3,3,1,192] f32, bk [192] f32 -> (wbias bf16, smats [128,512] bf16)."""
    g = _Geom(h, w, c)
    wfree = g.wfree
    if wb_compress:
        wb = np.zeros((128, 9 * c + wfree), np.float32)
        for dh in range(3):
            for dw in range(3):
                t = dh * 3 + dw
                wb[:, t * c:(t + 1) * c] = wk[dh, dw, 0][None, :]
        wb[:, 9 * c:9 * c + wfree] = np.tile(bk, g.wwin)[None, :]
    else:
        wb = np.zeros((128, 10 * wfree), np.float32)
        for dh in range(3):
            for dw in range(3):
                t = dh * 3 + dw
                pat = np.tile(wk[dh, dw, 0], g.wwin)
                wb[:, t * wfree:(t + 1) * wfree] = pat[None, :]
        wb[:, 9 * wfree:10 * wfree] = np.tile(bk, g.wwin)[None, :]

    sm = np.zeros((128, 4 * 128), np.float32)
    for i, dh in enumerate((-1, 0, 1)):
        for m in range(h):
            k = m + dh
            if 0 <= k < h:
                sm[k, i * 128 + m] = 1.0
    sm[0, 3 * 128:3 * 128 + h] = 1.0  # bias selector row
    return (wb.astype(ml_dtypes.bfloat16), sm.astype(ml_dtypes.bfloat16))


_DEFAULT_OPTS = dict(mm_order="dh", wb_compress=True, fuse_mults=True)

_NC_CACHE = {}


def kernel(x, w, b):
    x = np.ascontiguousarray(np.asarray(x, dtype=np.float32))
    wk = np.asarray(w, dtype=np.float32)
    bk = np.asarray(b, dtype=np.float32)
    assert x.shape == (B, H, W, C), x.shape

    has_bias = bool(np.any(bk != 0.0))
    key = ("nc", has_bias)
    if key not in _NC_CACHE:
        _NC_CACHE[key] = _build_module(opts=dict(_DEFAULT_OPTS, bias=has_bias))
    nc = _NC_CACHE[key]

    wbias, smats = _host_consts(wk, bk, wb_compress=_DEFAULT_OPTS["wb_compress"])
    in_maps = []
    for core in range(N_CORES):
        sh = x[core * B_SH:(core + 1) * B_SH]
        in_maps.append({"x": np.ascontiguousarray(sh), "wbias": wbias,
                        "smats": smats})
    res = run_bass_kernel_spmd(nc, in_maps, core_ids=list(range(N_CORES)))
    out = np.empty((B, H, W, C), np.float32)
    for core in range(N_CORES):
        out[core * B_SH:(core + 1) * B_SH] = res.results[core]["y"]
    return out



# revision 2
# speedup vs baseline: 3.3171x; 3.3171x over previous
"""Depthwise 3x3 conv (stride 1, SAME, depth_multiplier 1) on 8 trn2 NeuronCores.

Input  x [32, 112, 112, 192] f32, w [3, 3, 1, 192] f32, b [192] f32.
Output [32, 112, 112, 192] f32.

Strategy (v3 = v2 "banded-stationary" + 2D sharding):
  Cores = 2 image-groups x 4 channel-quarters -> each core owns 16 images
  x 48 channels (no cross-core communication).  Host casts x to bf16 and
  slices it per core; per (channel, w-tap kw) a banded 112x112 stationary
  S[k, m] = w[k-m+1, kw, c] folds all three h-taps into ONE accumulating
  PE matmul over moving tile x[h, (img4, w), c]:
      psum[h_out, (img, w_out)] += S_c_kw^T @ x[h_in, (img, w_out+kw-1), c]
  Per channel: 3 matmuls (kw = 1, 0, 2; SAME padding from the band/company
  col ranges).  144 stationaries resident (31.5KB/partition), x processed
  in 4-image full-W batches (42KB tiles, 3 rotating buffers) so DMA
  overlaps compute.  ScalarE/VectorE alternate evacuating psum -> bf16
  IN-PLACE into the x tile (channel-disjoint), which leaves a
  (w, c)-contiguous region for the store DMA.  Host casts bf16 back.
"""
import dataclasses

import numpy as np
import ml_dtypes

import concourse.bacc as bacc
import concourse.mybir as mybir
from concourse.bass_utils import run_bass_kernel_spmd
from concourse.tile import TileContext

F32 = mybir.dt.float32
BF16 = mybir.dt.bfloat16

B, H, W, C = 32, 112, 112, 192
N_CORES = 8
IMG_SPLIT = 1                # image-groups
CH_SPLIT = N_CORES // IMG_SPLIT
B_SH = B // IMG_SPLIT        # images per core (16)
C_SH = C // CH_SPLIT         # channels per core (48)
PAIR = 4                     # images per compute batch
SLOT = 512                   # psum f32 per channel slot (PAIR*W=448, padded)
GRP = 1                      # channels per psum tile (1 bank)
KW_ORDER = (1, 0, 2)         # w-tap emission order (center first: start=True)
NST = C_SH * 3

_DEFAULT_OPTS = dict(evac="alt", bias=False)


def _ap(t, off, dims):
    """AP with explicit [stride, count] dims; offset off in elements."""
    return dataclasses.replace(t[:, off:off + 1], ap=dims)


def _load_batch(nc, xt, x_b, pair):
    nc.sync.dma_start(
        out=xt[:, 0:pair * W * C_SH].rearrange("h (i cw) -> h i cw", i=pair),
        in_=x_b.rearrange("i h c w -> h i (c w)"))


def _batches(b_sh):
    """Batch sizes [2, 4, ..., 4, 2]: small first batch starts compute
    early; small last batch shrinks the unhideable store tail."""
    if b_sh <= PAIR:
        return [(0, b_sh)]
    out, p0 = [(0, 2)], 2
    while b_sh - p0 > 2 + PAIR:
        out.append((p0, PAIR))
        p0 += PAIR
    rem = b_sh - p0
    if rem > 2:
        out.append((p0, rem - 2))
        p0 += rem - 2
    out.append((p0, 2))
    return out


def _emit_batch(nc, wt_sb, xt, ot, pspool, x_b, y_b, opts, bias_sb=None,
                pair=PAIR):
    """One batch: 3*C_SH accumulating matmuls, dual-engine evac, store."""
    pitch = PAIR * W * C_SH          # allocated tile pitch (uniform)
    pfree = GRP * SLOT
    skip = opts.get("skip", ())

    for grp in range(C_SH // GRP):
        pt = pspool.tile([H, pfree], F32, tag="pt", name="pt")
        if "mm" not in skip:
            for jch in range(GRP):
                ch = grp * GRP + jch
                sl0 = jch * SLOT
                for j, kw in enumerate(KW_ORDER):
                    o0 = max(0, 1 - kw)
                    oe = W - 1 if kw == 2 else W
                    nw = oe - o0
                    t0 = o0 + kw - 1
                    st = (ch * 3 + j) % (NST if not opts.get("one_stat") else 1)
                    mov = _ap(xt, ch * W + t0,
                              [[pitch, H], [W * C_SH, pair], [1, nw]])
                    out = _ap(pt, sl0 + o0,
                              [[pfree, H], [W, pair], [1, nw]])
                    nc.tensor.matmul(
                        out, wt_sb[:, st * W:st * W + W], mov,
                        start=(j == 0), stop=(j == 2))
        if "evac" in skip:
            continue
        c0 = grp * GRP
        # Evac split by image-halves: ScalarE takes imgs [0, pair/2),
        # VectorE the rest.  The two dst regions are offset-disjoint in the
        # out tile, so the engines run fully in parallel with no WAW sems.
        hp = pair // 2
        for eng, i0 in ((nc.scalar, 0), (nc.vector, hp)):
            dst = _ap(ot, i0 * W * C_SH + c0 * W,
                      [[pitch, H], [W, GRP], [W * C_SH, hp], [1, W]])
            src = _ap(pt, i0 * W,
                      [[pfree, H], [SLOT, GRP], [W, hp], [1, W]])
            if opts.get("bias"):
                eng2 = nc.vector if eng is nc.scalar else nc.gpsimd
                eng2.scalar_tensor_tensor(
                    dst, src, 1.0,
                    _ap(bias_sb, c0,
                        [[C_SH, H], [1, GRP], [0, hp], [0, W]]),
                    mybir.AluOpType.mult, mybir.AluOpType.add)
            elif eng is nc.scalar:
                eng.activation(dst, src, mybir.ActivationFunctionType.Copy)
            else:
                eng.tensor_copy(dst, src)

    if "store" not in skip:
        # Store from the Activation engine queue (HWDGE): keeps the SP
        # queue free for loads, so a waiting store never blocks a load.
        src = _ap(ot, 0, [[pitch, H], [W * C_SH, pair], [1, W * C_SH]])
        nc.scalar.dma_start(
            out=y_b.rearrange("i h c w -> h i (c w)"), in_=src)


def _build_module(b_sh=B_SH, c_sh=C_SH, opts=None):
    opts = dict(_DEFAULT_OPTS, **(opts or {}))
    nc = bacc.Bacc("TRN2")
    x = nc.dram_tensor("x", [b_sh, H, c_sh, W], BF16, kind="ExternalInput")
    wt = nc.dram_tensor("wt", [H, NST * W], BF16, kind="ExternalInput")
    y = nc.dram_tensor("y", [b_sh, H, c_sh, W], BF16, kind="ExternalOutput")
    bi = (nc.dram_tensor("bi", [H, c_sh], F32, kind="ExternalInput")
          if opts.get("bias") else None)

    with TileContext(nc) as tc:
        with (
            tc.tile_pool(name="const", bufs=1) as cpool,
            tc.tile_pool(name="xp", bufs=opts.get("bufs", 2)) as xpool,
            tc.tile_pool(name="op", bufs=opts.get("obufs", 2)) as opool,
            tc.tile_pool(name="psum", bufs=opts.get("psbufs", 8),
                         space="PSUM") as pspool,
        ):
            batches = _batches(b_sh)
            wt_sb = cpool.tile([H, NST * W], BF16, tag="wt", name="wt")
            xt0 = xpool.tile([H, PAIR * W * c_sh], BF16, tag="xt", name="xt")
            _load_batch(nc, xt0, x[0:batches[0][1]], batches[0][1])
            for a in range(0, NST, 24):
                nc.sync.dma_start(out=wt_sb[:, a * W:(a + 24) * W],
                                  in_=wt[:, a * W:(a + 24) * W])
            bias_sb = None
            if opts.get("bias"):
                bias_sb = cpool.tile([H, c_sh], F32, tag="bi", name="bi")
                nc.sync.dma_start(out=bias_sb[:, :], in_=bi[:, :])
            xts = {0: xt0}
            for bi_, (p0, pr) in enumerate(batches):
                xt = xts.pop(bi_)
                if bi_ + 1 < len(batches):   # prefetch next batch's load
                    np0, npr = batches[bi_ + 1]
                    xtn = xpool.tile([H, PAIR * W * c_sh], BF16,
                                     tag="xt", name="xt")
                    _load_batch(nc, xtn, x[np0:np0 + npr], npr)
                    xts[bi_ + 1] = xtn
                ot = opool.tile([H, PAIR * W * c_sh], BF16, tag="ot", name="ot")
                _emit_batch(nc, wt_sb, xt, ot, pspool,
                            x[p0:p0 + pr], y[p0:p0 + pr],
                            opts, bias_sb, pair=pr)
    nc.compile()
    return nc


def _build_timing_module(iters=8, skip=(), opts=None):
    """Same per-batch pipeline in a HW loop over internal DRAM tensors."""
    opts = dict(_DEFAULT_OPTS, **(opts or {}), skip=skip)
    nc = bacc.Bacc("TRN2")
    x = nc.dram_tensor("xg", [B_SH, H, C_SH, W], BF16)
    y = nc.dram_tensor("yg", [B_SH, H, C_SH, W], BF16)
    yo = nc.dram_tensor("yo", [1, 8], F32, kind="ExternalOutput")

    with TileContext(nc) as tc:
        with (
            tc.tile_pool(name="const", bufs=1) as cpool,
            tc.tile_pool(name="xp", bufs=opts.get("bufs", 2)) as xpool,
            tc.tile_pool(name="op", bufs=opts.get("obufs", 2)) as opool,
            tc.tile_pool(name="stg", bufs=1) as spool,
            tc.tile_pool(name="psum", bufs=opts.get("psbufs", 8),
                         space="PSUM") as pspool,
        ):
            wt_sb = cpool.tile([H, NST * W], BF16, tag="wt", name="wt")
            nc.vector.memset(wt_sb[:, :], 0.01)
            zt = spool.tile([H, W * C_SH], BF16, tag="zt", name="zt")
            nc.vector.memset(zt[:, :], 0.5)
            for img in range(B_SH):
                nc.sync.dma_start(
                    out=x[img].rearrange("h c w -> h (c w)"), in_=zt[:, :])
            with tc.For_i(0, iters) as _:
                for p0, pr in _batches(B_SH):
                    xt = xpool.tile([H, PAIR * W * C_SH], BF16,
                                    tag="xt", name="xt")
                    ot = opool.tile([H, PAIR * W * C_SH], BF16,
                                    tag="ot", name="ot")
                    if "load" not in opts.get("skip", ()):
                        _load_batch(nc, xt, x[p0:p0 + pr], pr)
                    _emit_batch(nc, wt_sb, xt, ot, pspool,
                                x[p0:p0 + pr], y[p0:p0 + pr], opts, pair=pr)
            of = spool.tile([1, 8], F32, tag="of", name="of")
            nc.vector.memset(of[:, :], 0.0)
            nc.sync.dma_start(out=yo[:, :], in_=of[:1, :8])
    nc.compile()
    return nc


def _host_consts(wk, bk, ch0):
    """Banded stationaries for channels [ch0, ch0+C_SH) -> [112, NST*112]."""
    st_arr = np.zeros((H, NST, W), np.float32)
    m = np.arange(W)
    for ci in range(C_SH):
        c = ch0 + ci
        for j, kw in enumerate(KW_ORDER):
            st = ci * 3 + j
            for kh in range(3):
                k = m + kh - 1
                sel = (k >= 0) & (k < H)
                st_arr[k[sel], st, m[sel]] = wk[kh, kw, 0, c]
    bias = np.broadcast_to(bk[None, ch0:ch0 + C_SH], (H, C_SH)).astype(
        np.float32).copy()
    return (np.ascontiguousarray(st_arr.reshape(H, NST * W))
            .astype(ml_dtypes.bfloat16), bias)


_NC_CACHE = {}


def kernel(x, w, b):
    x = np.asarray(x, dtype=np.float32)
    wk = np.asarray(w, dtype=np.float32)
    bk = np.asarray(b, dtype=np.float32)
    assert x.shape == (B, H, W, C), x.shape

    has_bias = bool(np.any(bk != 0.0))
    key = ("nc", has_bias)
    if key not in _NC_CACHE:
        _NC_CACHE[key] = _build_module(opts=dict(bias=has_bias))
    nc = _NC_CACHE[key]

    xb = x.astype(ml_dtypes.bfloat16)
    in_maps = []
    for core in range(N_CORES):
        ig, cg = divmod(core, CH_SPLIT)
        isl = slice(ig * B_SH, (ig + 1) * B_SH)
        csl = slice(cg * C_SH, (cg + 1) * C_SH)
        stats, bias = _host_consts(wk, bk, cg * C_SH)
        m = {"x": np.ascontiguousarray(
                 xb[isl, :, :, csl].transpose(0, 1, 3, 2)), "wt": stats}
        if has_bias:
            m["bi"] = bias
        in_maps.append(m)
    res = run_bass_kernel_spmd(nc, in_maps, core_ids=list(range(N_CORES)))
    out = np.empty((B, H, W, C), np.float32)
    for core in range(N_CORES):
        ig, cg = divmod(core, CH_SPLIT)
        isl = slice(ig * B_SH, (ig + 1) * B_SH)
        csl = slice(cg * C_SH, (cg + 1) * C_SH)
        out[isl, :, :, csl] = res.results[core]["y"].transpose(0, 1, 3, 2)
    return out


# revision 9
# speedup vs baseline: 4.0048x; 1.2073x over previous
"""Depthwise 3x3 conv (stride 1, SAME, depth_multiplier 1) on 8 trn2 NeuronCores.

Input  x [32, 112, 112, 192] f32, w [3, 3, 1, 192] f32, b [192] f32.
Output [32, 112, 112, 192] f32.

Strategy (v3 "banded-stationary" + channel sharding):
  Cores = 8 channel-slices -> each core owns all 32 images x 24 channels
  (no cross-core communication).  Host casts x to bf16 and pre-transposes
  each core's slice to channel-planar [img, h, c, w] so every on-chip
  access is contiguous.  Per (channel, w-tap kw) a banded 112x112
  stationary S[k, m] = w[k-m+1, kw, c] folds all three h-taps into ONE
  accumulating PE matmul over moving tile x[h, (img, c, w)]:
      psum[h_out, (img, w_out)] += S_c_kw^T @ x[h_in, (img, c, w_out+kw-1)]
  Per channel: 3 matmuls (kw = 1, 0, 2; SAME padding falls out of the
  band row range and per-tap column ranges).  72 stationaries stay
  resident (15.75KB/partition); images run in batches [2,4,...,4,2]
  (small edge batches shrink pipeline fill/drain) with double-buffered
  x and out tiles so load DMA (SP queue), PE, evac, and store DMA
  (Activation queue) all overlap.  ScalarE and VectorE evacuate
  psum->bf16 in parallel on disjoint image-halves of the out tile.
  Host casts the bf16 result back to f32 and re-transposes.
"""
import dataclasses

import numpy as np
import ml_dtypes

import concourse.bacc as bacc
import concourse.mybir as mybir
from concourse.bass_utils import run_bass_kernel_spmd
from concourse.tile import TileContext

F32 = mybir.dt.float32
BF16 = mybir.dt.bfloat16

B, H, W, C = 32, 112, 112, 192
N_CORES = 8
IMG_SPLIT = 1                # image-groups
CH_SPLIT = N_CORES // IMG_SPLIT
B_SH = B // IMG_SPLIT        # images per core (16)
C_SH = C // CH_SPLIT         # channels per core (48)
PAIR = 4                     # images per compute batch
SLOT = 512                   # psum f32 per channel slot (PAIR*W=448, padded)
GRP = 1                      # channels per psum tile (1 bank)
KW_ORDER = (1, 0, 2)         # w-tap emission order (center first: start=True)
NST = C_SH * 3

_DEFAULT_OPTS = dict(evac="alt", bias=False)


def _ap(t, off, dims):
    """AP with explicit [stride, count] dims; offset off in elements."""
    return dataclasses.replace(t[:, off:off + 1], ap=dims)


def _load_batch(nc, xt, x_b, pair):
    nc.sync.dma_start(
        out=xt[:, 0:pair * W * C_SH].rearrange("h (i cw) -> h i cw", i=pair),
        in_=x_b.rearrange("i h c w -> h i (c w)"))


def _batches(b_sh):
    """Batch sizes [1, 2, 4, ..., 4, 2, 1]: small edge batches shrink the
    unhideable pipeline fill (first load) and drain (last store)."""
    if b_sh <= PAIR:
        return [(0, b_sh)]
    sizes = [2] + [PAIR] * max(0, (b_sh - 4) // PAIR) + [2]
    rem = b_sh - sum(sizes)
    if rem:
        sizes.insert(1, rem)
    assert sum(sizes) == b_sh, (sizes, b_sh)
    out, p0 = [], 0
    for sz in sizes:
        out.append((p0, sz))
        p0 += sz
    return out


def _emit_batch(nc, wt_sb, xt, ot, pspool, x_b, y_b, opts, bias_sb=None,
                pair=PAIR):
    """One batch: 3*C_SH accumulating matmuls, dual-engine evac, store."""
    pitch = PAIR * W * C_SH          # allocated tile pitch (uniform)
    pfree = GRP * SLOT
    skip = opts.get("skip", ())

    for grp in range(C_SH // GRP):
        pt = pspool.tile([H, pfree], F32, tag="pt", name="pt")
        if "mm" not in skip:
            for jch in range(GRP):
                ch = grp * GRP + jch
                sl0 = jch * SLOT
                for j, kw in enumerate(KW_ORDER):
                    o0 = max(0, 1 - kw)
                    oe = W - 1 if kw == 2 else W
                    nw = oe - o0
                    t0 = o0 + kw - 1
                    st = (ch * 3 + j) % (NST if not opts.get("one_stat") else 1)
                    mov = _ap(xt, ch * W + t0,
                              [[pitch, H], [W * C_SH, pair], [1, nw]])
                    out = _ap(pt, sl0 + o0,
                              [[pfree, H], [W, pair], [1, nw]])
                    nc.tensor.matmul(
                        out, wt_sb[:, st * W:st * W + W], mov,
                        start=(j == 0), stop=(j == 2))
        if "evac" in skip:
            continue
        c0 = grp * GRP
        # Evac split by image-halves: ScalarE takes imgs [0, pair/2),
        # VectorE the rest.  The two dst regions are offset-disjoint in the
        # out tile, so the engines run fully in parallel with no WAW sems.
        h0 = (pair + 1) // 2
        for eng, i0, hp in ((nc.scalar, 0, h0), (nc.vector, h0, pair - h0)):
            if hp == 0:
                continue
            dst = _ap(ot, i0 * W * C_SH + c0 * W,
                      [[pitch, H], [W, GRP], [W * C_SH, hp], [1, W]])
            src = _ap(pt, i0 * W,
                      [[pfree, H], [SLOT, GRP], [W, hp], [1, W]])
            if opts.get("bias"):
                eng2 = nc.vector if eng is nc.scalar else nc.gpsimd
                eng2.scalar_tensor_tensor(
                    dst, src, 1.0,
                    _ap(bias_sb, c0,
                        [[C_SH, H], [1, GRP], [0, hp], [0, W]]),
                    mybir.AluOpType.mult, mybir.AluOpType.add)
            elif eng is nc.scalar:
                eng.activation(dst, src, mybir.ActivationFunctionType.Copy)
            else:
                eng.tensor_copy(dst, src)

    if "store" not in skip:
        # Store from the Activation engine queue (HWDGE): keeps the SP
        # queue free for loads, so a waiting store never blocks a load.
        src = _ap(ot, 0, [[pitch, H], [W * C_SH, pair], [1, W * C_SH]])
        nc.scalar.dma_start(
            out=y_b.rearrange("i h c w -> h i (c w)"), in_=src)


def _build_module(b_sh=B_SH, c_sh=C_SH, opts=None):
    opts = dict(_DEFAULT_OPTS, **(opts or {}))
    nc = bacc.Bacc("TRN2")
    x = nc.dram_tensor("x", [b_sh, H, c_sh, W], BF16, kind="ExternalInput")
    wt = nc.dram_tensor("wt", [H, NST * W], BF16, kind="ExternalInput")
    y = nc.dram_tensor("y", [b_sh, H, c_sh, W], BF16, kind="ExternalOutput")
    bi = (nc.dram_tensor("bi", [H, c_sh], F32, kind="ExternalInput")
          if opts.get("bias") else None)

    with TileContext(nc) as tc:
        with (
            tc.tile_pool(name="const", bufs=1) as cpool,
            tc.tile_pool(name="xp", bufs=opts.get("bufs", 2)) as xpool,
            tc.tile_pool(name="op", bufs=opts.get("obufs", 2)) as opool,
            tc.tile_pool(name="psum", bufs=opts.get("psbufs", 8),
                         space="PSUM") as pspool,
        ):
            batches = _batches(b_sh)
            wt_sb = cpool.tile([H, NST * W], BF16, tag="wt", name="wt")
            xt0 = xpool.tile([H, PAIR * W * c_sh], BF16, tag="xt", name="xt")
            _load_batch(nc, xt0, x[0:batches[0][1]], batches[0][1])
            for a in range(0, NST, 24):
                nc.sync.dma_start(out=wt_sb[:, a * W:(a + 24) * W],
                                  in_=wt[:, a * W:(a + 24) * W])
            bias_sb = None
            if opts.get("bias"):
                bias_sb = cpool.tile([H, c_sh], F32, tag="bi", name="bi")
                nc.sync.dma_start(out=bias_sb[:, :], in_=bi[:, :])
            xts = {0: xt0}
            for bi_, (p0, pr) in enumerate(batches):
                xt = xts.pop(bi_)
                if bi_ + 1 < len(batches):   # prefetch next batch's load
                    np0, npr = batches[bi_ + 1]
                    xtn = xpool.tile([H, PAIR * W * c_sh], BF16,
                                     tag="xt", name="xt")
                    _load_batch(nc, xtn, x[np0:np0 + npr], npr)
                    xts[bi_ + 1] = xtn
                ot = opool.tile([H, PAIR * W * c_sh], BF16, tag="ot", name="ot")
                _emit_batch(nc, wt_sb, xt, ot, pspool,
                            x[p0:p0 + pr], y[p0:p0 + pr],
                            opts, bias_sb, pair=pr)
    nc.compile()
    return nc


def _build_timing_module(iters=8, skip=(), opts=None):
    """Same per-batch pipeline in a HW loop over internal DRAM tensors."""
    opts = dict(_DEFAULT_OPTS, **(opts or {}), skip=skip)
    nc = bacc.Bacc("TRN2")
    x = nc.dram_tensor("xg", [B_SH, H, C_SH, W], BF16)
    y = nc.dram_tensor("yg", [B_SH, H, C_SH, W], BF16)
    yo = nc.dram_tensor("yo", [1, 8], F32, kind="ExternalOutput")

    with TileContext(nc) as tc:
        with (
            tc.tile_pool(name="const", bufs=1) as cpool,
            tc.tile_pool(name="xp", bufs=opts.get("bufs", 2)) as xpool,
            tc.tile_pool(name="op", bufs=opts.get("obufs", 2)) as opool,
            tc.tile_pool(name="stg", bufs=1) as spool,
            tc.tile_pool(name="psum", bufs=opts.get("psbufs", 8),
                         space="PSUM") as pspool,
        ):
            wt_sb = cpool.tile([H, NST * W], BF16, tag="wt", name="wt")
            nc.vector.memset(wt_sb[:, :], 0.01)
            zt = spool.tile([H, W * C_SH], BF16, tag="zt", name="zt")
            nc.vector.memset(zt[:, :], 0.5)
            for img in range(B_SH):
                nc.sync.dma_start(
                    out=x[img].rearrange("h c w -> h (c w)"), in_=zt[:, :])
            with tc.For_i(0, iters) as _:
                for p0, pr in _batches(B_SH):
                    xt = xpool.tile([H, PAIR * W * C_SH], BF16,
                                    tag="xt", name="xt")
                    ot = opool.tile([H, PAIR * W * C_SH], BF16,
                                    tag="ot", name="ot")
                    if "load" not in opts.get("skip", ()):
                        _load_batch(nc, xt, x[p0:p0 + pr], pr)
                    _emit_batch(nc, wt_sb, xt, ot, pspool,
                                x[p0:p0 + pr], y[p0:p0 + pr], opts, pair=pr)
            of = spool.tile([1, 8], F32, tag="of", name="of")
            nc.vector.memset(of[:, :], 0.0)
            nc.sync.dma_start(out=yo[:, :], in_=of[:1, :8])
    nc.compile()
    return nc


def _host_consts(wk, bk, ch0):
    """Banded stationaries for channels [ch0, ch0+C_SH) -> [112, NST*112]."""
    st_arr = np.zeros((H, NST, W), np.float32)
    m = np.arange(W)
    for ci in range(C_SH):
        c = ch0 + ci
        for j, kw in enumerate(KW_ORDER):
            st = ci * 3 + j
            for kh in range(3):
                k = m + kh - 1
                sel = (k >= 0) & (k < H)
                st_arr[k[sel], st, m[sel]] = wk[kh, kw, 0, c]
    bias = np.broadcast_to(bk[None, ch0:ch0 + C_SH], (H, C_SH)).astype(
        np.float32).copy()
    return (np.ascontiguousarray(st_arr.reshape(H, NST * W))
            .astype(ml_dtypes.bfloat16), bias)


_NC_CACHE = {}


def kernel(x, w, b):
    x = np.asarray(x, dtype=np.float32)
    wk = np.asarray(w, dtype=np.float32)
    bk = np.asarray(b, dtype=np.float32)
    assert x.shape == (B, H, W, C), x.shape

    has_bias = bool(np.any(bk != 0.0))
    key = ("nc", has_bias)
    if key not in _NC_CACHE:
        _NC_CACHE[key] = _build_module(opts=dict(bias=has_bias))
    nc = _NC_CACHE[key]

    xb = x.astype(ml_dtypes.bfloat16)
    in_maps = []
    for core in range(N_CORES):
        ig, cg = divmod(core, CH_SPLIT)
        isl = slice(ig * B_SH, (ig + 1) * B_SH)
        csl = slice(cg * C_SH, (cg + 1) * C_SH)
        stats, bias = _host_consts(wk, bk, cg * C_SH)
        m = {"x": np.ascontiguousarray(
                 xb[isl, :, :, csl].transpose(0, 1, 3, 2)), "wt": stats}
        if has_bias:
            m["bi"] = bias
        in_maps.append(m)
    res = run_bass_kernel_spmd(nc, in_maps, core_ids=list(range(N_CORES)))
    out = np.empty((B, H, W, C), np.float32)
    for core in range(N_CORES):
        ig, cg = divmod(core, CH_SPLIT)
        isl = slice(ig * B_SH, (ig + 1) * B_SH)
        csl = slice(cg * C_SH, (cg + 1) * C_SH)
        out[isl, :, :, csl] = res.results[core]["y"].transpose(0, 1, 3, 2)
    return out
